# revision 75
# baseline (speedup 1.0000x reference)
"""DiT forward on 8 TRN2 NeuronCores — data-parallel over batch (4 imgs/core).

v2: fp8e4 DoubleRow matmuls (2 k-tiles/instr at 0.5 cyc/row) for the big
GEMMs (qkv, v, out-proj, mlp1, mlp2, ada, and the partition-0 half of
attn@V + softmax sums — DoubleRow requires dst partition 0, so the
offset-64 head of each pair uses plain fp8 matmuls). Weights are
host-quantized to fp8e4 with a x32 scale folded back out (x1/32) in each
PSUM-drain pass. Activations feeding fp8 matmuls (hmod, oT, hmid, p, vsb,
scT) are written as fp8e4 directly by their producing pass. Attention scores
stay bf16 (no DoubleRow possible at 64-deep contraction); LN stats matmuls
stay bf16; patchify + final head stay bf16 for accuracy. Residual stream
tokT stays fp32; rstd = Exp(-0.5*Ln(var+eps)) keeps the ACT engine on one
activation table per layer half (a post-compile Belady pass re-places the
1283ns table loads off the critical path). Elementwise passes are spread
across ACT/DVE/Pool via the CFG engine knobs (GPSIMD cannot touch PSUM or
run scalar_tensor_tensor on HW — those stay on ACT/DVE).

Measured: 1,625,039 ns TimelineSim (baseline 3,113,270), HW rel err 1.55e-2.
"""
import math
import numpy as np

CORES = 8
B, C, IMG, PP = 32, 4, 32, 2
D, H, L = 768, 12, 12
HD = D // H          # 64
HID = 4 * D          # 3072
NCLS = 10
NP_ = (IMG // PP) ** 2   # 256 patches/img
NIMG = B // CORES        # 4 imgs per core
NTOK = NIMG * NP_        # 1024 tokens per core
KT = D // 128            # 6 feature tiles
KT2 = HID // 128         # 24
PDIM = C * PP * PP       # 16
WS = 32.0                # fp8 weight scale
IWS = 1.0 / WS

_NC_CACHE = {}
REGIONS = []   # (instr_id_start, label) markers recorded during build

# Engine assignment per elementwise pass ('act'/'dve'/'pool'), cycled by
# instruction index within the pass. Tuned against TimelineSim.
CFG = {
    "copy":      ['pool', 'dve', 'act', 'pool', 'dve', 'pool'],
    "sq":        ['dve', 'pool', 'dve', 'pool', 'dve', 'dve'],
    "t1stt":     ['dve', 'dve', 'dve', 'dve', 'pool', 'pool'],
    "t1mul":     ['dve', 'dve', 'dve', 'dve', 'dve', 'dve'],
    "mod1":      ['pool', 'dve'],
    "mod2":      ['act', 'dve', 'pool'],
    "modF":      ['act'],
    "negmean":   ['pool'],
    "qkv_drain": ['act', 'dve', 'dve'],
    "v_drain":   ['act', 'dve'],
    "oT_mul":    ['dve', 'pool'],
    "outp_drain": ['act'],
    "outp_resid": ['dve'],
    "m2_drain":  ['dve', 'act'],
    "m2_resid":  ['dve'],
}


def _mark(nc, label):
    REGIONS.append((nc.next_id(), label))


def _build():
    import concourse.bass as bass
    import concourse.tile as tile
    from concourse import bacc, mybir
    from contextlib import ExitStack

    f32 = mybir.dt.float32
    bf16 = mybir.dt.bfloat16
    fp8 = mybir.dt.float8e4
    AF = mybir.ActivationFunctionType
    OP = mybir.AluOpType
    DR = mybir.MatmulPerfMode.DoubleRow

    nc = bacc.Bacc("TRN2", target_bir_lowering=False, debug=False,
                   num_devices=CORES)

    def din(name, shape, dt=f32):
        return nc.dram_tensor(name, list(shape), dt, kind="ExternalInput").ap()

    # ---- per-core inputs ----
    xpT_d = din("xpT", [PDIM, NTOK], bf16)
    t4_d = din("t4", [1, NIMG])
    oneh_d = din("onehot", [NCLS + 1, NIMG], fp8)
    # ---- shared constants / weights (lhsT layout [128, kt, ...]) ----
    emb_d = din("emb", [1, D])
    posT_d = din("posT", [128, KT, NP_])
    wconv_d = din("wconv", [PDIM, D], bf16)
    wqk_d = din("wqk", [128, KT, L, 2 * D], fp8)
    wv_d = din("wv", [128, KT, L, D], fp8)
    wo_d = din("wo", [128, KT, L, D], fp8)
    wada_d = din("wada", [128, KT, L, 6 * D], fp8)
    wm1_d = din("wm1", [128, KT, L, HID], fp8)
    wm2_d = din("wm2", [128, KT2, L, D], fp8)
    wtm1_d = din("wtm1", [128, KT, HID], fp8)
    wtm2_d = din("wtm2", [128, KT2, D], fp8)
    wcls_d = din("wcls", [NCLS + 1, D], fp8)
    wfa_d = din("wfa", [128, KT, 2 * D], fp8)
    wfl_d = din("wfl", [128, KT, PDIM], bf16)
    bconv_d = din("bconv", [128, KT])
    bqk_d = din("bqk", [128, L, 2 * KT])
    bo_d = din("bo", [128, L, KT])          # outp_b with v-bias folded in
    bada_d = din("bada", [128, L, 6 * KT])
    bm1_d = din("bm1", [128, L, KT2])
    bm2_d = din("bm2", [128, L, KT])
    btm1_d = din("btm1", [128, KT2])
    btm2_d = din("btm2", [128, KT])
    bfa_d = din("bfa", [128, 2 * KT])
    bfl_d = din("bfl", [PDIM, 1])
    out_d = nc.dram_tensor("out", [PDIM, NTOK], f32, kind="ExternalOutput").ap()

    NH = 2               # token-column halves (matmul N<=512, mlp n-split)
    NCH = NTOK // NH     # 512

    with tile.TileContext(nc) as tc:
        ctx = ExitStack()
        with ctx:
            consts = ctx.enter_context(tc.tile_pool(name="consts", bufs=1))
            wpool = ctx.enter_context(tc.tile_pool(name="wpool", bufs=7))
            bft = ctx.enter_context(tc.tile_pool(name="bft", bufs=2))
            stat = ctx.enter_context(tc.tile_pool(name="stat", bufs=2))
            ppool = ctx.enter_context(tc.tile_pool(name="ppool", bufs=3))
            rspool = ctx.enter_context(tc.tile_pool(name="rspool", bufs=2))
            adap = ctx.enter_context(tc.tile_pool(name="adap", bufs=2))
            mm = ctx.enter_context(tc.tile_pool(name="mm", bufs=4, space="PSUM"))
            att = ctx.enter_context(tc.tile_pool(name="att", bufs=2, space="PSUM"))

            # ---- persistent sbuf ----
            tokT = consts.tile([128, KT, NTOK], f32, tag="tokT")
            qkT = consts.tile([128, 2 * KT, NTOK], bf16, tag="qkT")
            vsb = consts.tile([128, NTOK // 128, D], fp8, tag="vsb")
            oT = consts.tile([128, KT, NTOK], fp8, tag="oT")
            hmid = consts.tile([128, KT2, NTOK], fp8, tag="hmid")
            ones8 = consts.tile([128, 2, 64], fp8, tag="ones8")
            nc.vector.memset(ones8, 1.0)
            ones_inv = consts.tile([128, 128], bf16, tag="ones_inv")
            nc.vector.memset(ones_inv, 1.0 / D)
            epst = consts.tile([128, 1], f32, tag="epst")
            nc.vector.memset(epst, 1e-6)
            zerot = consts.tile([128, 1], f32, tag="zerot")
            nc.vector.memset(zerot, 0.0)
            pihalf = consts.tile([128, 1], f32, tag="pihalf")
            nc.vector.memset(pihalf, math.pi / 2)

            # ---- load constants ----
            posT = consts.tile([128, KT, NP_], f32, tag="posT")
            nc.sync.dma_start(out=posT, in_=posT_d)
            biases = {}
            for nm, d_ap, shape in [
                ("bconv", bconv_d, [128, KT]), ("bqk", bqk_d, [128, L, 2 * KT]),
                ("bo", bo_d, [128, L, KT]), ("bada", bada_d, [128, L, 6 * KT]),
                ("bm1", bm1_d, [128, L, KT2]), ("bm2", bm2_d, [128, L, KT]),
                ("btm1", btm1_d, [128, KT2]), ("btm2", btm2_d, [128, KT]),
                ("bfa", bfa_d, [128, 2 * KT]), ("bfl", bfl_d, [PDIM, 1]),
            ]:
                tl = consts.tile(shape, f32, name=nm, tag=nm)
                nc.sync.dma_start(out=tl, in_=d_ap)
                biases[nm] = tl
            xpT = consts.tile([PDIM, NTOK], bf16, tag="io16", bufs=1)
            nc.sync.dma_start(out=xpT, in_=xpT_d)
            t4 = consts.tile([1, NIMG], f32, tag="t4")
            nc.sync.dma_start(out=t4, in_=t4_d)
            oneh = consts.tile([NCLS + 1, NIMG], fp8, tag="oneh")
            nc.sync.dma_start(out=oneh, in_=oneh_d)
            emb = consts.tile([1, D], f32, tag="emb")
            nc.sync.dma_start(out=emb, in_=emb_d)
            wconv = consts.tile([PDIM, D], bf16, tag="wconv")
            nc.sync.dma_start(out=wconv, in_=wconv_d)
            wcls = consts.tile([NCLS + 1, D], fp8, tag="wcls")
            nc.sync.dma_start(out=wcls, in_=wcls_d)

            _mark(nc, 'cpath')
            # ---- conditioning path: temb -> silu-mlp -> + cls -> silu ----
            # (tmlp weights fp8 x32; temb/h1 in fp8 — modulation-path precision
            # is gate-damped so 4% is fine)
            tembT = consts.tile([128, KT, NIMG], fp8, tag="tembT")
            for cch in range(KT):
                ps = att.tile([128, NIMG], f32, name="ps_e", tag="att")
                nc.tensor.matmul(ps, lhsT=emb[:, cch * 128:(cch + 1) * 128],
                                 rhs=t4, start=True, stop=True)
                sbias = zerot if cch < KT // 2 else pihalf
                with nc.allow_low_precision(reason="temb fp8"):
                    nc.scalar.activation(tembT[:, cch, :], ps, AF.Sin,
                                         bias=sbias)
            h1T = consts.tile([128, KT2, NIMG], fp8, tag="h1T")
            for chunk in range(4):
                wt = wpool.tile([128, KT, 768], fp8, tag="w", name="wtm1_t")
                nc.sync.dma_start(
                    out=wt, in_=wtm1_d[:, :, chunk * 768:(chunk + 1) * 768])
                for m in range(6):
                    ps = att.tile([128, NIMG], f32, name="ps_h1", tag="att")
                    for kk in range(KT // 2):
                        nc.tensor.matmul(
                            ps, lhsT=wt[:, 2 * kk:2 * kk + 2,
                                        m * 128:(m + 1) * 128],
                            rhs=tembT[:, 2 * kk:2 * kk + 2, :],
                            start=(kk == 0), stop=(kk == KT // 2 - 1),
                            perf_mode=DR)
                    mi = chunk * 6 + m
                    with nc.allow_low_precision(reason="h1 fp8"):
                        nc.scalar.activation(h1T[:, mi, :], ps, AF.Silu,
                                             scale=IWS,
                                             bias=biases["btm1"][:, mi:mi + 1])
            # scT = silu(c) in fp8 (feeds fp8 DoubleRow ada matmuls)
            scT = consts.tile([128, KT, NIMG], fp8, tag="scT")
            wt2 = []
            for ck in range(4):
                w_ = wpool.tile([128, 6, D], fp8, tag="w", name=f"wtm2_{ck}")
                nc.sync.dma_start(out=w_, in_=wtm2_d[:, ck * 6:(ck + 1) * 6, :])
                wt2.append(w_)
            for m in range(KT):
                ps = att.tile([128, NIMG], f32, name="ps_c", tag="att")
                for kk in range(KT2 // 2):
                    wt = wt2[kk // 3]
                    k2 = (kk % 3) * 2
                    nc.tensor.matmul(ps, lhsT=wt[:, k2:k2 + 2,
                                                 m * 128:(m + 1) * 128],
                                     rhs=h1T[:, 2 * kk:2 * kk + 2, :],
                                     start=(kk == 0), stop=False, perf_mode=DR)
                nc.tensor.matmul(ps, lhsT=wcls[:, m * 128:(m + 1) * 128],
                                 rhs=oneh, start=False, stop=True)
                with nc.allow_low_precision(reason="scT fp8 for ada matmuls"):
                    nc.scalar.activation(scT[:, m, :], ps, AF.Silu, scale=IWS,
                                         bias=biases["btm2"][:, m:m + 1])

            _mark(nc, 'patchify')
            # ---- patchify: tokT = wconv.T @ xpT + bconv + pos ----
            for m in range(KT):
                for nh in range(NH):
                    sl = slice(nh * NCH, (nh + 1) * NCH)
                    ps = mm.tile([128, NCH], f32, name="ps_conv", tag="mm")
                    nc.tensor.matmul(ps, lhsT=wconv[:, m * 128:(m + 1) * 128],
                                     rhs=xpT[:, sl], start=True, stop=True)
                    nc.scalar.activation(tokT[:, m, sl], ps, AF.Identity,
                                         bias=biases["bconv"][:, m:m + 1])
                for img in range(NIMG):
                    sl = slice(img * NP_, (img + 1) * NP_)
                    nc.vector.tensor_add(tokT[:, m, sl], tokT[:, m, sl],
                                         posT[:, m, :])

            # ---- helpers ----
            def compute_ada(w_dram_cols, ncols, bias2d, name):
                """adaT[:, m, img] = (ada_w.T @ silu(c)) / WS + bias.

                fp8 DoubleRow matmuls, all m-tiles into one PSUM tile,
                single DVE drain with broadcast bias."""
                nch = ncols // 128
                adaT = adap.tile([128, nch, NIMG], f32, tag="ada", name=name)
                ps = att.tile([128, nch, NIMG], f32, name=f"ps_{name}",
                              tag="att")
                nchunks = (ncols + 767) // 768
                for chunk in range(nchunks):
                    c0 = chunk * 768
                    cw = min(768, ncols - c0)
                    wt = wpool.tile([128, KT, cw], fp8, tag="wa", bufs=3,
                                    name=f"{name}_w")
                    nc.sync.dma_start(out=wt, in_=w_dram_cols(c0, cw))
                    for m in range(cw // 128):
                        mi = c0 // 128 + m
                        for kk in range(KT // 2):
                            nc.tensor.matmul(
                                ps[:, mi, :],
                                lhsT=wt[:, 2 * kk:2 * kk + 2,
                                        m * 128:(m + 1) * 128],
                                rhs=scT[:, 2 * kk:2 * kk + 2, :],
                                start=(kk == 0), stop=(kk == KT // 2 - 1),
                                perf_mode=DR)
                nc.vector.scalar_tensor_tensor(
                    adaT, in0=ps, scalar=IWS,
                    in1=bias2d[:, :, None].broadcast_to([128, nch, NIMG]),
                    op0=OP.mult, op1=OP.add)
                return adaT

            def pick(key, i):
                lst = CFG[key]
                return lst[i % len(lst)]

            ENG = {'dve': nc.vector, 'pool': nc.gpsimd}

            def t_copy(eng, out, in_):
                if eng == 'act':
                    nc.scalar.copy(out, in_)
                else:
                    ENG[eng].tensor_copy(out=out, in_=in_)

            def t_ts(eng, out, in0, s1, s2):
                """out = in0*s1 + s2 (per-partition scalars)."""
                if eng == 'act':
                    nc.scalar.activation(out, in0, AF.Identity,
                                         bias=s2, scale=s1)
                else:
                    ENG[eng].tensor_scalar(out, in0=in0, scalar1=s1,
                                           scalar2=s2, op0=OP.mult, op1=OP.add)

            def ln_mod(sh_ch, sc_ch, adaT, name, out_dt, modkey):
                """hmod = LN(tokT)*(1+sc)+sh -> out_dt (fp8 inner / bf16 fin).

                Stats from a bf16 copy. Inner layers compute
                rstd = Exp(-0.5*Ln(var+eps)) so the ACT engine stays on the
                natural_log_exp table (shared with attention's Exp) instead
                of paying a 1283ns table switch for Abs_reciprocal_sqrt."""
                tokbf = bft.tile([128, KT, NTOK], bf16, tag="bft",
                                 name=f"{name}_xbf")
                sq = bft.tile([128, KT, NTOK], bf16, tag="bft", name=f"{name}_sq")
                t1 = bft.tile([128, KT, NTOK], bf16, tag="bft", name=f"{name}_t1")
                hmod = bft.tile([128, KT, NTOK], out_dt, tag="bfth",
                                name=f"{name}_hmod")
                # per-half copies/squares, both halves emitted upfront so
                # nh=1's inputs stream while nh=0 waits on its serial ACT
                # rstd chain. sq reads tokT directly (not the bf16 copy) so
                # copy and sq run in parallel off the residual write.
                for nh in range(NH):
                    sl = slice(nh * NCH, (nh + 1) * NCH)
                    for kc in range(KT):
                        e = pick('sq', nh * KT + kc)
                        if e == 'act':
                            nc.scalar.activation(sq[:, kc, sl],
                                                 tokT[:, kc, sl], AF.Square)
                        else:
                            ENG[e].tensor_mul(sq[:, kc, sl], tokT[:, kc, sl],
                                              tokT[:, kc, sl])
                        t_copy(pick('copy', nh * KT + kc), tokbf[:, kc, sl],
                               tokT[:, kc, sl])
                for nh in range(NH):
                    sl = slice(nh * NCH, (nh + 1) * NCH)
                    stt2 = att.tile([128, 2, NCH], f32, name=f"{name}_stats",
                                    tag="att")
                    meanb = stt2[:, 0, :]
                    sqmb = stt2[:, 1, :]
                    for kc in range(KT):
                        nc.tensor.matmul(meanb, lhsT=ones_inv,
                                         rhs=tokbf[:, kc, sl],
                                         start=(kc == 0), stop=(kc == KT - 1))
                    for kc in range(KT):
                        nc.tensor.matmul(sqmb, lhsT=ones_inv,
                                         rhs=sq[:, kc, sl],
                                         start=(kc == 0), stop=(kc == KT - 1))
                    m2 = stat.tile([128, NCH], f32, tag="st", name=f"{name}_m2")
                    nc.scalar.activation(m2, meanb, AF.Square)
                    var = stat.tile([128, NCH], f32, tag="st", name=f"{name}_var")
                    nc.vector.scalar_tensor_tensor(var, in0=sqmb, scalar=1.0,
                                                   in1=m2, op0=OP.mult,
                                                   op1=OP.subtract)
                    rstd = stat.tile([128, NCH], bf16, tag="st",
                                     name=f"{name}_rstd")
                    with nc.allow_low_precision(reason="rstd bf16"):
                        if out_dt == fp8:
                            lnv = stat.tile([128, NCH], f32, tag="st",
                                            name=f"{name}_lnv")
                            nc.scalar.activation(lnv, var, AF.Ln, bias=epst)
                            nc.scalar.activation(rstd, lnv, AF.Exp, scale=-0.5)
                            # bf16 -mean so t1 = tokbf + negmean runs in the
                            # DVE 2x mode
                            negmean = stat.tile([128, NCH], bf16, tag="st",
                                                name=f"{name}_nm")
                            # reads PSUM: ACT or DVE only
                            if pick('negmean', nh) == 'act':
                                nc.scalar.activation(negmean, meanb, AF.Copy,
                                                     scale=-1.0)
                            else:
                                nc.vector.tensor_scalar_mul(negmean, meanb,
                                                            -1.0)
                        else:
                            nc.scalar.activation(rstd, var,
                                                 AF.Abs_reciprocal_sqrt,
                                                 bias=epst)
                    for kc in range(KT):
                        i6 = nh * KT + kc
                        if out_dt == fp8:
                            ENG[pick('t1stt', i6)].tensor_add(
                                t1[:, kc, sl], tokbf[:, kc, sl], negmean)
                        else:
                            # reads PSUM (meanb) — GPSIMD can't access PSUM
                            nc.vector.scalar_tensor_tensor(
                                t1[:, kc, sl], in0=meanb, scalar=-1.0,
                                in1=tokT[:, kc, sl], op0=OP.mult, op1=OP.add)
                        ENG[pick('t1mul', i6)].tensor_mul(
                            t1[:, kc, sl], t1[:, kc, sl], rstd)
                        for i2 in range(2):
                            img = 2 * nh + i2
                            isl2 = slice(img * NP_, (img + 1) * NP_)
                            with nc.allow_low_precision(reason="hmod low-prec"):
                                t_ts(pick(modkey, i6 * 2 + i2),
                                     hmod[:, kc, isl2], t1[:, kc, isl2],
                                     adaT[:, sc_ch + kc, img:img + 1],
                                     adaT[:, sh_ch + kc, img:img + 1])
                return hmod

            def drain(eng, out_ap, ps, bias_ap, n):
                """out = ps*IWS + bias (per-partition bias) on eng.

                Reads PSUM, so only ACT/DVE are legal (GPSIMD cannot access
                PSUM on hardware)."""
                if eng == 'act':
                    nc.scalar.activation(out_ap, ps, AF.Identity,
                                         bias=bias_ap, scale=IWS)
                else:
                    nc.vector.scalar_tensor_tensor(out_ap, in0=ps, scalar=IWS,
                                                   in1=bias_ap.broadcast_to(
                                                       [128, n]),
                                                   op0=OP.mult, op1=OP.add)

            # ---- transformer layers ----
            def layer_ada(li):
                _mark(nc, f'L{li}.ada')
                a = compute_ada(
                    lambda c0, cw, li=li: wada_d[:, :, li, c0:c0 + cw],
                    6 * D, biases["bada"][:, li, :], f"ada{li}")
                # chunks: [0:6]=sh1 [6:12]=sc1 [12:18]=g1
                #         [18:24]=sh2 [24:30]=sc2 [30:36]=g2
                nc.vector.tensor_scalar_add(a[:, 6:12, :], a[:, 6:12, :], 1.0)
                nc.gpsimd.tensor_scalar_add(a[:, 24:30, :], a[:, 24:30, :], 1.0)
                return a

            adaT_next = layer_ada(0)
            for li in range(L):
                adaT = adaT_next

                _mark(nc, f'L{li}.ln1')
                # ===== attention branch =====
                hmod = ln_mod(0, 6, adaT, f"l{li}a", fp8, 'mod1')
                _mark(nc, f'L{li}.qkv')
                wqk_t = []
                for ck in range(2):
                    w_ = wpool.tile([128, KT, D], fp8, tag="w",
                                    name=f"wqk{li}_{ck}")
                    nc.sync.dma_start(out=w_,
                                      in_=wqk_d[:, :, li, ck * D:(ck + 1) * D])
                    wqk_t.append(w_)
                wv_t = wpool.tile([128, KT, D], fp8, tag="w", name=f"wv{li}")
                nc.sync.dma_start(out=wv_t, in_=wv_d[:, :, li, :])
                # v projection first: token-major (v-bias folded into outp
                # bias) so attention@V never waits on it
                for mt in range(NTOK // 128):
                    for c0, cw in ((0, 512), (512, 256)):
                        ps = mm.tile([128, cw], f32, name="ps_v", tag="mm")
                        for kk in range(KT // 2):
                            nc.tensor.matmul(
                                ps,
                                lhsT=hmod[:, 2 * kk:2 * kk + 2,
                                          mt * 128:(mt + 1) * 128],
                                rhs=wv_t[:, 2 * kk:2 * kk + 2, c0:c0 + cw],
                                start=(kk == 0), stop=(kk == KT // 2 - 1),
                                perf_mode=DR)
                        with nc.allow_low_precision(reason="vsb fp8"):
                            e = pick('v_drain', mt * 2 + (c0 > 0))
                            # GPSIMD cannot access PSUM on HW
                            if e == 'act':
                                nc.scalar.activation(vsb[:, mt, c0:c0 + cw],
                                                     ps, AF.Copy, scale=IWS)
                            else:
                                nc.vector.tensor_scalar_mul(
                                    vsb[:, mt, c0:c0 + cw], ps, IWS)
                wo_t = wpool.tile([128, KT, D], fp8, tag="w", name=f"wo{li}")
                nc.sync.dma_start(out=wo_t, in_=wo_d[:, :, li, :])

                def qkv_mm(m):
                    wt = wqk_t[m // 6]
                    msl = slice((m % 6) * 128, (m % 6) * 128 + 128)
                    for nh in range(NH):
                        sl = slice(nh * NCH, (nh + 1) * NCH)
                        ps = mm.tile([128, NCH], f32, name="ps_qk", tag="mm")
                        for kk in range(KT // 2):
                            nc.tensor.matmul(
                                ps, lhsT=wt[:, 2 * kk:2 * kk + 2, msl],
                                rhs=hmod[:, 2 * kk:2 * kk + 2, sl],
                                start=(kk == 0), stop=(kk == KT // 2 - 1),
                                perf_mode=DR)
                        with nc.allow_low_precision(reason="qkT bf16"):
                            drain(pick('qkv_drain', m * 2 + nh),
                                  qkT[:, m, sl], ps,
                                  biases["bqk"][:, li, m:m + 1], NCH)

                def attn_hp(hp):
                    # attention for one head-pair over all imgs; both subs'
                    # scores in one 2-bank psum -> a single 1024-col exp
                    for img in range(NIMG):
                        isl = slice(img * NP_, (img + 1) * NP_)
                        # one 2-bank psum slot: cols 0:256 = attn@V pair,
                        # cols 256:512 = per-head exp-sums (S) at the head's
                        # partition offset
                        os_ps = mm.tile([128, 2 * NP_], f32, name="ps_os",
                                        tag="mm")
                        sc_ps = att.tile([128, 4, NP_], f32, name="ps_sc",
                                         tag="att")
                        for sub in range(2):
                            po = 64 * sub
                            q_sl = qkT[po:po + 64, hp, isl]
                            for kc in range(2):
                                kb = img * NP_ + kc * 128
                                k_sl = qkT[po:po + 64, KT + hp, kb:kb + 128]
                                nc.tensor.matmul(
                                    sc_ps[:, 2 * sub + kc, :],
                                    lhsT=k_sl, rhs=q_sl,
                                    start=True, stop=True)
                        p_sb = ppool.tile([128, 4, NP_], fp8, tag="p",
                                          name="p_sb")
                        with nc.allow_low_precision(reason="p fp8"):
                            nc.scalar.activation(p_sb, sc_ps, AF.Exp)
                        for sub in range(2):
                            hh = 2 * hp + sub
                            po = 64 * sub
                            p_sl = p_sb[:, 2 * sub:2 * sub + 2, :]
                            if sub == 0:
                                # DoubleRow only valid at dst partition 0
                                # (s3d3_mm_valid_dst_partition)
                                nc.tensor.matmul(
                                    os_ps[po:po + 64, NP_:2 * NP_],
                                    lhsT=ones8, rhs=p_sl,
                                    start=True, stop=True, perf_mode=DR)
                                nc.tensor.matmul(
                                    os_ps[po:po + 64, 0:NP_],
                                    lhsT=vsb[:, 2 * img:2 * img + 2,
                                             hh * 64:hh * 64 + 64],
                                    rhs=p_sl,
                                    start=True, stop=True, perf_mode=DR)
                            else:
                                for kc in range(2):
                                    nc.tensor.matmul(
                                        os_ps[po:po + 64, NP_:2 * NP_],
                                        lhsT=ones8[:, 0, :],
                                        rhs=p_sb[:, 2 + kc, :],
                                        start=(kc == 0), stop=(kc == 1))
                                for kc in range(2):
                                    nc.tensor.matmul(
                                        os_ps[po:po + 64, 0:NP_],
                                        lhsT=vsb[:, 2 * img + kc,
                                                 hh * 64:hh * 64 + 64],
                                        rhs=p_sb[:, 2 + kc, :],
                                        start=(kc == 0), stop=(kc == 1))
                        rs = rspool.tile([128, NP_], bf16, tag="rs", name="rs")
                        with nc.allow_low_precision(reason="softmax 1/S bf16"):
                            nc.vector.reciprocal(rs, os_ps[:, NP_:2 * NP_])
                            # reads PSUM: DVE only (tensor_mul needs 2 tensors)
                            nc.vector.tensor_mul(oT[:, hp, isl],
                                                 os_ps[:, 0:NP_], rs)

                # interleave: qkv (q,k) pairs head-pair-major, attention for
                # head-pair hp-1 between pairs so exps start early
                _mark(nc, f'L{li}.attn')
                for hp in range(H // 2):
                    qkv_mm(hp)
                    qkv_mm(KT + hp)
                    if hp >= 1:
                        attn_hp(hp - 1)
                attn_hp(H // 2 - 1)
                # out projection + gated residual
                _mark(nc, f'L{li}.outp')
                for m in range(KT):
                    for nh in range(NH):
                        sl = slice(nh * NCH, (nh + 1) * NCH)
                        ps = mm.tile([128, NCH], f32, name="ps_o", tag="mm")
                        for kk in range(KT // 2):
                            nc.tensor.matmul(
                                ps, lhsT=wo_t[:, 2 * kk:2 * kk + 2,
                                              m * 128:(m + 1) * 128],
                                rhs=oT[:, 2 * kk:2 * kk + 2, sl],
                                start=(kk == 0), stop=(kk == KT // 2 - 1),
                                perf_mode=DR)
                        tmp = stat.tile([128, NCH], bf16, tag="st", name="tmp_o")
                        with nc.allow_low_precision(reason="resid tmp bf16"):
                            drain(pick('outp_drain', m * 2 + nh), tmp, ps,
                                  biases["bo"][:, li, m:m + 1], NCH)
                        for i2 in range(2):
                            img = nh * 2 + i2
                            slo = slice(img * NP_, (img + 1) * NP_)
                            sli = slice(i2 * NP_, (i2 + 1) * NP_)
                            e = pick('outp_resid', (m * 2 + nh) * 2 + i2)
                            ENG[e].scalar_tensor_tensor(
                                tokT[:, m, slo], in0=tmp[:, sli],
                                scalar=adaT[:, 12 + m, img:img + 1],
                                in1=tokT[:, m, slo], op0=OP.mult, op1=OP.add)

                _mark(nc, f'L{li}.ln2')
                # ===== mlp branch =====
                hmod = ln_mod(18, 24, adaT, f"l{li}m", fp8, 'mod2')
                _mark(nc, f'L{li}.mlp')
                w2 = []
                for ck in range(4):
                    w_ = wpool.tile([128, 6, D], fp8, tag="w",
                                    name=f"wm2_{li}_{ck}")
                    nc.sync.dma_start(
                        out=w_, in_=wm2_d[:, ck * 6:(ck + 1) * 6, li, :])
                    w2.append(w_)
                # mlp1 chunk-outer: each w1 chunk DMA'd once, used for both
                # token halves; hmid holds the full token range. m-tiles are
                # drained in pairs: one 2-bank psum -> a single 1024-col gelu.
                for chunk in range(4):
                    w1 = wpool.tile([128, KT, 768], fp8, tag="w",
                                    name=f"wm1_{li}_{chunk}")
                    nc.sync.dma_start(
                        out=w1,
                        in_=wm1_d[:, :, li, chunk * 768:(chunk + 1) * 768])
                    for m in range(6):
                        mi = chunk * 6 + m
                        ps = att.tile([128, 2, NCH], f32, name="ps_m1",
                                      tag="att")
                        for nh in range(NH):
                            sl = slice(nh * NCH, (nh + 1) * NCH)
                            for kk in range(KT // 2):
                                nc.tensor.matmul(
                                    ps[:, nh, :],
                                    lhsT=w1[:, 2 * kk:2 * kk + 2,
                                            m * 128:(m + 1) * 128],
                                    rhs=hmod[:, 2 * kk:2 * kk + 2, sl],
                                    start=(kk == 0),
                                    stop=(kk == KT // 2 - 1),
                                    perf_mode=DR)
                        with nc.allow_low_precision(reason="hmid fp8"):
                            nc.scalar.activation(
                                hmid[:, mi, :], ps, AF.Gelu, scale=IWS,
                                bias=biases["bm1"][:, li, mi:mi + 1])
                if li + 1 < L:
                    adaT_next = layer_ada(li + 1)
                # mlp2 m-outer over both halves: residual for feature tile m
                # completes across all imgs early, so the next layer's ln1
                # pipelines in behind it
                for m in range(KT):
                    for nh in range(NH):
                        sl = slice(nh * NCH, (nh + 1) * NCH)
                        ps = mm.tile([128, NCH], f32, name="ps_m2", tag="mm")
                        for kk in range(KT2 // 2):
                            wt = w2[kk // 3]
                            k2 = (kk % 3) * 2
                            nc.tensor.matmul(
                                ps, lhsT=wt[:, k2:k2 + 2,
                                            m * 128:(m + 1) * 128],
                                rhs=hmid[:, 2 * kk:2 * kk + 2, sl],
                                start=(kk == 0), stop=(kk == KT2 // 2 - 1),
                                perf_mode=DR)
                        tmp = stat.tile([128, NCH], bf16, tag="st", name="tmp_m")
                        with nc.allow_low_precision(reason="resid tmp bf16"):
                            drain(pick('m2_drain', m * 2 + nh), tmp, ps,
                                  biases["bm2"][:, li, m:m + 1], NCH)
                        for i2 in range(2):     # 2 imgs per token-half
                            img = nh * 2 + i2
                            slo = slice(img * NP_, (img + 1) * NP_)
                            sli = slice(i2 * NP_, (i2 + 1) * NP_)
                            e = pick('m2_resid', (m * 2 + nh) * 2 + i2)
                            ENG[e].scalar_tensor_tensor(
                                tokT[:, m, slo], in0=tmp[:, sli],
                                scalar=adaT[:, 30 + m, img:img + 1],
                                in1=tokT[:, m, slo], op0=OP.mult, op1=OP.add)

            _mark(nc, 'final')
            # ---- final adaLN + linear head (bf16 for accuracy) ----
            adaF = compute_ada(lambda c0, cw: wfa_d[:, :, c0:c0 + cw],
                               2 * D, biases["bfa"], "adaF")
            nc.vector.tensor_scalar_add(adaF[:, 6:12, :], adaF[:, 6:12, :], 1.0)
            hmodF = ln_mod(0, 6, adaF, "fin", bf16, 'modF')
            wfl_t = wpool.tile([128, KT, PDIM], bf16, tag="w", name="wfl_t")
            nc.sync.dma_start(out=wfl_t, in_=wfl_d)
            out_sb = consts.tile([PDIM, NTOK], f32, tag="io16", bufs=1)
            for nh in range(NH):
                sl = slice(nh * NCH, (nh + 1) * NCH)
                ps = mm.tile([PDIM, NCH], f32, name="ps_fin", tag="mm")
                for kc in range(KT):
                    nc.tensor.matmul(ps, lhsT=wfl_t[:, kc, :],
                                     rhs=hmodF[:, kc, sl],
                                     start=(kc == 0), stop=(kc == KT - 1))
                nc.scalar.activation(out_sb[:, sl], ps, AF.Identity,
                                     bias=biases["bfl"])
            nc.sync.dma_start(out=out_d, in_=out_sb)

    nc.compile()
    _replace_act_table_loads(nc, mybir)
    return nc


def _replace_act_table_loads(nc, mybir):
    """Re-place InstLoadActFuncSet optimally (Belady furthest-next-miss).

    The built-in greedy pass picks the first table containing each function
    (natural_log for Ln, exp_and_others for Exp), paying two 1283ns loads per
    rstd = Exp(-0.5*Ln(var)) even though natural_log_exp_and_others serves
    both (plus attention's Exp). Loads carry no sync_info, so removing and
    re-inserting them only changes the ACT queue program order."""
    from concourse.hw_specs import get_activation_tables
    tables = list(get_activation_tables(nc.m.arch).items())
    for blk in nc.main_func.blocks:
        insts = blk.instructions
        kept = []
        funcs_at = []          # indices (into kept) of ACT activations
        for i in insts:
            if isinstance(i, mybir.InstLoadActFuncSet):
                si = i.sync_info
                if si is None or (not len(si.on_wait) and not len(si.on_update)):
                    continue   # drop; we re-place below
            kept.append(i)
        seq = [(idx, i.func) for idx, i in enumerate(kept)
               if isinstance(i, mybir.InstActivation)
               and i.engine == mybir.EngineType.Activation]
        cur = None             # index into tables
        inserts = []           # (kept_idx, table_idx)
        for si, (idx, f) in enumerate(seq):
            if cur is not None and f in tables[cur][1]:
                continue
            best, best_len = None, -1
            for ti, (_, fset) in enumerate(tables):
                if f not in fset:
                    continue
                n = 0
                for _, f2 in seq[si + 1:]:
                    if f2 in fset:
                        n += 1
                    else:
                        break
                if n > best_len:
                    best, best_len = ti, n
            # hoist the load to just after the last activation whose func is
            # NOT in the new table — table-agnostic ops (Identity/Square/...)
            # run under either table, so the 1283ns load executes during a
            # busy stretch instead of on the critical path before e.g. Ln
            pos = idx
            for pj, pf in reversed(seq[:si]):
                if pf not in tables[best][1]:
                    pos = pj + 1
                    break
            else:
                pos = 0 if cur is None else idx
            cur = best
            inserts.append((pos, best))
        inserts.sort(key=lambda x: x[0])
        for idx, ti in reversed(inserts):
            ld = mybir.InstLoadActFuncSet(
                name=nc.get_next_instruction_name(),
                act_func_set_id=ti, ins=[], outs=[])
            ld.engine = mybir.EngineType.Activation
            nc.register_instruction(ld)
            kept.insert(idx, ld)
        del insts[:]
        insts.extend(kept)


def _get_nc():
    if "nc" not in _NC_CACHE:
        _NC_CACHE["nc"] = _build()
    return _NC_CACHE["nc"]


def _host_prep(inputs):
    """Host-side layout prep: shard batch, fold scales/biases, lhsT layouts."""
    import ml_dtypes
    BF = ml_dtypes.bfloat16
    F8 = ml_dtypes.float8_e4m3
    inp = {k: np.asarray(v) for k, v in inputs.items()}
    x = inp["x"].astype(np.float32)
    t = inp["t"].astype(np.float32)
    lab = np.asarray(inp["class_label"]).astype(np.int64)

    h = IMG // PP
    xp = x.reshape(B, C, h, PP, h, PP).transpose(0, 2, 4, 1, 3, 5)
    xp = xp.reshape(B, NP_, PDIM)

    # pos embed (constant)
    pos = np.arange(NP_, dtype=np.float32)[:, None]
    dim = np.arange(0, D, 2, dtype=np.float32)
    ang = pos / np.power(10000.0, dim / np.float32(D))
    pe = np.zeros((NP_, D), dtype=np.float32)
    pe[:, 0::2] = np.sin(ang)
    pe[:, 1::2] = np.cos(ang)
    posT = np.ascontiguousarray(pe.T.reshape(KT, 128, NP_).transpose(1, 0, 2))

    half = D // 2
    emb1 = np.exp(np.arange(half, dtype=np.float32)
                  * -(math.log(10000.0) / (half - 1))).astype(np.float32)
    emb = np.ascontiguousarray(np.concatenate([emb1, emb1])[None, :])

    scale = np.float32(1.0 / math.sqrt(HD))
    qkv_w = inp["qkv_w"].astype(np.float32).copy()
    qkv_b = inp["qkv_b"].astype(np.float32).copy()
    qkv_w[:, :, :D] *= scale
    qkv_b[:, :D] *= scale
    bv = qkv_b[:, 2 * D:]
    outp_w = inp["outp_w"].astype(np.float32)
    bo_eff = inp["outp_b"].astype(np.float32) + np.einsum("ld,ldo->lo", bv, outp_w)

    def lhsT_L(w, dt=BF, ws=1.0):  # [L, K, M] -> [128, K/128, L, M]
        L_, K_, M_ = w.shape
        return np.ascontiguousarray(
            (w * ws).reshape(L_, K_ // 128, 128, M_).transpose(2, 1, 0, 3)
        ).astype(dt)

    def lhsT_1(w, dt=BF, ws=1.0):  # [K, M] -> [128, K/128, M]
        K_, M_ = w.shape
        return np.ascontiguousarray(
            (w * ws).reshape(K_ // 128, 128, M_).transpose(1, 0, 2)).astype(dt)

    def bias_L(b):        # [L, M] -> [128, L, M/128]
        L_, M_ = b.shape
        return np.ascontiguousarray(
            b.reshape(L_, M_ // 128, 128).transpose(2, 0, 1)).astype(np.float32)

    def bias_1(b):        # [M] -> [128, M/128]
        M_ = b.shape[0]
        return np.ascontiguousarray(b.reshape(M_ // 128, 128).T).astype(np.float32)

    shared = {
        "emb": emb, "posT": posT,
        "wconv": np.ascontiguousarray(
            inp["conv_w"].astype(np.float32).reshape(D, PDIM).T).astype(BF),
        "wqk": lhsT_L(qkv_w[:, :, :2 * D], F8, WS),
        "wv": lhsT_L(qkv_w[:, :, 2 * D:], F8, WS),
        "wo": lhsT_L(outp_w, F8, WS),
        "wada": lhsT_L(inp["ada_w"].astype(np.float32), F8, WS),
        "wm1": lhsT_L(inp["mlp_w1"].astype(np.float32), F8, WS),
        "wm2": lhsT_L(inp["mlp_w2"].astype(np.float32), F8, WS),
        "wtm1": lhsT_1(inp["tmlp_w1"].astype(np.float32), F8, WS),
        "wtm2": lhsT_1(inp["tmlp_w2"].astype(np.float32), F8, WS),
        "wcls": (inp["cls_emb"].astype(np.float32) * WS).astype(F8),
        "wfa": lhsT_1(inp["fin_ada_w"].astype(np.float32), F8, WS),
        "wfl": lhsT_1(inp["fin_lin_w"].astype(np.float32)),
        "bconv": bias_1(inp["conv_b"].astype(np.float32)),
        "bqk": bias_L(qkv_b[:, :2 * D]),
        "bo": bias_L(bo_eff),
        "bada": bias_L(inp["ada_b"].astype(np.float32)),
        "bm1": bias_L(inp["mlp_b1"].astype(np.float32)),
        "bm2": bias_L(inp["mlp_b2"].astype(np.float32)),
        "btm1": bias_1(inp["tmlp_b1"].astype(np.float32)),
        "btm2": bias_1(inp["tmlp_b2"].astype(np.float32)),
        "bfa": bias_1(inp["fin_ada_b"].astype(np.float32)),
        "bfl": np.ascontiguousarray(
            inp["fin_lin_b"].astype(np.float32)[:, None]),
    }
    in_maps = []
    for core in range(CORES):
        sl = slice(core * NIMG, (core + 1) * NIMG)
        xpT = np.ascontiguousarray(xp[sl].reshape(NTOK, PDIM).T)
        onehot = np.zeros((NCLS + 1, NIMG), np.float32)
        for i, lv in enumerate(lab[sl]):
            onehot[int(lv), i] = 1.0
        m = dict(shared)
        m["xpT"] = xpT.astype(BF)
        m["t4"] = np.ascontiguousarray(t[sl][None, :])
        m["onehot"] = onehot.astype(F8)
        in_maps.append(m)
    return in_maps


def _unpatchify(res_core):
    """[16, 1024] -> [NIMG, C, IMG, IMG]"""
    h = IMG // PP
    r = res_core.reshape(PP, PP, C, NIMG, h, h)       # (pi, pj, c, img, hh, ww)
    return np.ascontiguousarray(
        r.transpose(3, 2, 4, 0, 5, 1).reshape(NIMG, C, IMG, IMG))


def kernel(**inputs):
    from concourse.bass_utils import run_bass_kernel_spmd
    nc = _get_nc()
    in_maps = _host_prep(inputs)
    res = run_bass_kernel_spmd(nc, in_maps, core_ids=list(range(CORES)))
    out = np.concatenate(
        [_unpatchify(res.results[c]["out"]) for c in range(CORES)], axis=0)
    return out.astype(np.float32)
